# revision 28
# baseline (speedup 1.0000x reference)
"""Trainium2 Bass kernel for nn_AtteMatchLay (multi-perspective cosine matching).

Math (per flattened row n, perspective p):
    dot[n,p] = sum_d r[n,d]*m[n,d]*w2[p,d]
    n1s[n,p] = sum_d r[n,d]^2 * w2[p,d]        (w2 = weight**2)
    n2s[n,p] = sum_d m[n,d]^2 * w2[p,d]
    cos[n,p] = dot / (sqrt(n1s)*sqrt(n2s))

Strategy: data-parallel over the flattened N=16*512=8192 rows across 8 cores
(1024 rows each), contraction dim D on SBUF partitions (6 blocks of 128).

This kernel is DVE+ACT throughput-bound: the three elementwise products are
~13.5 engine-seconds over the two usable engines (GpSimd shares an
exclusive-lock SBUF port pair with DVE's 2-port mode and would stall it).
Everything is organized to (a) start those engines as early as possible,
(b) keep ops at full 1024 width for efficiency, (c) keep the post-stream
tail chain short.

  * r and m are packed interleaved into ONE DRAM tensor; each SP-queue
    trigger lands a matched (r,m) block pair (ACT queue must stay free:
    its sequencer serializes triggers against engine ops). Block 0 is the
    very first trigger so products start ~1.5us earlier; block 5 streams
    last as four quarter chunks so the final chain is short.
  * Products: rm+mm on DVE (2x bf16), rr on ACT (Square). Full width for
    b0..b4, quarter width for b5.
  * PSUM: two column groups x {dot,n1,n2} in their own banks (matmul
    start=True resets a whole bank, so groups never share banks).
  * Epilogue per group: u1,u2 = ARSQRT(n1s,n2s) on ACT (PSUM-direct,
    bf16 out), t = u1*u2 (DVE bf16 2x), cos = dot*t (bf16 out), DMA out.
    A dummy ARSQRT issued before any Square pins the one ACT table
    (abs_reciprocal_sqrt_and_small serves both) during the DMA-wait
    window instead of a 1.3us reload on the tail.
"""

import sys

if "/opt/trn_rl_repo" not in sys.path:
    sys.path.insert(0, "/opt/trn_rl_repo")

import numpy as np

# ---- problem constants (hardcoded per contract) ----
BSZ, SL, D, MP = 16, 512, 768, 20
N = BSZ * SL           # 8192 flattened rows
NCORES = 8
NSH = N // NCORES      # 1024 rows per core
P = 128                # SBUF partitions
NB = D // P            # 6 d-blocks
NBF = NB - 1           # blocks streamed full width
G = 2                  # PSUM column groups
GW = NSH // G          # 512
Q = 4                  # tail quarters of block 5
QW = NSH // Q          # 256

_CACHE = {}


def _build():
    import concourse.tile as tile
    from concourse import bacc, mybir

    f32 = mybir.dt.float32
    bf16 = mybir.dt.bfloat16
    nc = bacc.Bacc(None, target_bir_lowering=False)

    xD = nc.dram_tensor("xD", [P, 2 * NB * NSH], bf16, kind="ExternalInput")
    w2D = nc.dram_tensor("w2D", [P, NB * MP], bf16, kind="ExternalInput")
    out = nc.dram_tensor("out", [MP, NSH], bf16, kind="ExternalOutput")

    SQ = mybir.ActivationFunctionType.Square
    ARSQRT = mybir.ActivationFunctionType.Abs_reciprocal_sqrt
    MUL = mybir.AluOpType.mult

    with tile.TileContext(nc) as tc:
        with (
            tc.tile_pool(name="const", bufs=1) as const,
            tc.tile_pool(name="inp", bufs=1) as inp,
            tc.tile_pool(name="prod", bufs=3) as prod,
            tc.tile_pool(name="epi", bufs=1) as epi,
            tc.tile_pool(name="psum", bufs=1, space="PSUM") as psum,
        ):
            w2_sb = const.tile([P, NB, MP], bf16, tag="w2")
            x_sb = inp.tile([P, NB, 2, NSH], bf16, tag="x")
            bias_b = const.tile([P, 1], bf16, tag="bias_b")
            bias_f = const.tile([MP, 1], f32, tag="bias_f")
            dum = const.tile([MP, 1], f32, tag="dum")
            nc.gpsimd.memset(bias_b[:], 0.0)
            nc.gpsimd.memset(bias_f[:], 0.0)
            nc.gpsimd.memset(dum[:], 1.0)

            nc.scalar.activation(dum[:], dum[:], ARSQRT, bias=bias_f[:])

            # ---- DMA triggers (all SP queue, stream order) ----
            def ld(b, c0, c1):
                # chunk holds [r-cols | m-cols] of block b, columns c0:c1
                w = c1 - c0
                off = ld.off
                nc.sync.dma_start(
                    out=x_sb[:, b, :, c0:c1],
                    in_=xD[:, off : off + 2 * w].rearrange("p (t n) -> p t n", t=2),
                )
                ld.off = off + 2 * w

            ld.off = 0
            ld(0, 0, NSH)                       # block 0 first: earliest compute
            nc.sync.dma_start(
                out=w2_sb[:], in_=w2D[:, :].rearrange("p (b q) -> p b q", b=NB)
            )
            for b in range(1, NBF):
                ld(b, 0, NSH)
            # block 5 in column halves: 2KB descriptors (quarter-pair chunks
            # would drop to 1KB descriptors and halve the tail stream rate)
            ld(NB - 1, 0, GW)
            ld(NB - 1, GW, NSH)

            # ---- PSUM accumulators: per-group banks ----
            dot_ps, n1_ps, n2_ps = [], [], []
            for g in range(G):
                dps = psum.tile([MP, GW], f32, tag=f"dot{g}")
                n1p = psum.tile([MP, GW], f32, tag=f"n1{g}")
                n2p = psum.tile([MP, GW], f32, tag=f"n2{g}")
                dot_ps.append(dps)
                n1_ps.append(n1p)
                n2_ps.append(n2p)

            u1 = epi.tile([MP, NSH], bf16, tag="u1")
            u2 = epi.tile([MP, NSH], bf16, tag="u2")
            tt = epi.tile([MP, NSH], bf16, tag="tt")
            cos = epi.tile([MP, NSH], bf16, tag="cos")

            # ---- blocks 0..4: full-width products + 6 matmuls each ----
            for b in range(NBF):
                rsl = x_sb[:, b, 0, :]
                msl = x_sb[:, b, 1, :]
                rm = prod.tile([P, NSH], bf16, tag="rm")
                rr = prod.tile([P, NSH], bf16, tag="rr")
                mm = prod.tile([P, NSH], bf16, tag="mm")
                nc.vector.tensor_tensor(rm[:], rsl, msl, MUL)
                nc.scalar.activation(rr[:], rsl, SQ, bias=bias_b[:])
                nc.vector.tensor_tensor(mm[:], msl, msl, MUL)
                w2b = w2_sb[:, b, :]
                st = b == 0
                for g in range(G):
                    gsl = slice(g * GW, (g + 1) * GW)
                    nc.tensor.matmul(dot_ps[g][:], w2b, rm[:, gsl], start=st, stop=False)
                    nc.tensor.matmul(n1_ps[g][:], w2b, rr[:, gsl], start=st, stop=False)
                    nc.tensor.matmul(n2_ps[g][:], w2b, mm[:, gsl], start=st, stop=False)

            # ---- block 5: group 0's half at quarter granularity so its
            # arsqrts start mid-tail and drain off ACT before group 1's
            # chain needs the engine; group 1 stays half-granular. ----
            rm5 = prod.tile([P, NSH], bf16, tag="rm5")
            rr5 = prod.tile([P, NSH], bf16, tag="rr5")
            mm5 = prod.tile([P, NSH], bf16, tag="mm5")
            w2b5 = w2_sb[:, NB - 1, :]
            b5 = NB - 1
            QW = GW // 2

            def prod_mm(c0, c1, g, oc0, oc1):
                cs = slice(c0, c1)
                rq = x_sb[:, b5, 0, cs]
                mq = x_sb[:, b5, 1, cs]
                nc.vector.tensor_tensor(rm5[:, cs], rq, mq, MUL)
                nc.scalar.activation(rr5[:, cs], rq, SQ, bias=bias_b[:])
                nc.vector.tensor_tensor(mm5[:, cs], mq, mq, MUL)
                osl = slice(oc0, oc1)
                kw = dict(start=False, stop=True, skip_group_check=True)
                nc.tensor.matmul(dot_ps[g][:, osl], w2b5, rm5[:, cs], **kw)
                nc.tensor.matmul(n1_ps[g][:, osl], w2b5, rr5[:, cs], **kw)
                nc.tensor.matmul(n2_ps[g][:, osl], w2b5, mm5[:, cs], **kw)

            def epi(gs, g, osl):
                # cos = (dot*u1)*u2: the first DVE multiply overlaps ACT's
                # second arsqrt; the final multiply is cheap bf16 2x.
                nc.scalar.activation(u1[:, gs], n1_ps[g][:, osl], ARSQRT, bias=bias_f[:])
                nc.scalar.activation(u2[:, gs], n2_ps[g][:, osl], ARSQRT, bias=bias_f[:])
                nc.vector.tensor_tensor(tt[:, gs], dot_ps[g][:, osl], u1[:, gs], MUL)
                nc.vector.tensor_tensor(cos[:, gs], tt[:, gs], u2[:, gs], MUL)

            prod_mm(0, QW, 0, 0, QW)          # g0 quarter 0
            prod_mm(QW, GW, 0, QW, GW)        # g0 quarter 1
            prod_mm(GW, NSH, 1, 0, GW)        # g1 half
            epi(slice(0, QW), 0, slice(0, QW))
            epi(slice(QW, GW), 0, slice(QW, GW))
            nc.sync.dma_start(out=out[:, 0:GW], in_=cos[:, 0:GW])
            epi(slice(GW, NSH), 1, slice(0, GW))
            nc.sync.dma_start(out=out[:, GW:NSH], in_=cos[:, GW:NSH])

    nc.finalize()
    return nc


def get_nc():
    if "nc" not in _CACHE:
        _CACHE["nc"] = _build()
    return _CACHE["nc"]


def _pack_pair(r2d, m2d):
    # [1024 rows, 768] f32 x2 -> [128, 12288] bf16 in stream-chunk order:
    # [r-b0|m-b0] ... [r-b4|m-b4], then block 5 as 4 quarter chunks.
    import ml_dtypes

    rt = r2d.T.reshape(NB, P, NSH)  # [b, p, n]
    mt = m2d.T.reshape(NB, P, NSH)
    parts = []
    for b in range(NBF):
        parts.append(rt[b])
        parts.append(mt[b])
    for g in range(G):
        gs = slice(g * GW, (g + 1) * GW)
        parts.append(rt[NB - 1][:, gs])
        parts.append(mt[NB - 1][:, gs])
    x = np.concatenate(parts, axis=1)
    return np.ascontiguousarray(x.astype(ml_dtypes.bfloat16))


def make_in_maps(repres, max_att, weight):
    import ml_dtypes

    r = np.ascontiguousarray(repres, dtype=np.float32).reshape(N, D)
    m = np.ascontiguousarray(max_att, dtype=np.float32).reshape(N, D)
    w2t = (weight.astype(np.float32) ** 2).T  # [D, MP]
    w2d = np.ascontiguousarray(
        w2t.reshape(NB, P, MP).transpose(1, 0, 2).reshape(P, NB * MP)
        .astype(ml_dtypes.bfloat16)
    )
    in_maps = []
    for c in range(NCORES):
        rows = slice(c * NSH, (c + 1) * NSH)
        in_maps.append({"xD": _pack_pair(r[rows], m[rows]), "w2D": w2d})
    return in_maps


def gather(results):
    cols = np.concatenate(
        [results[c]["out"].astype(np.float32) for c in range(NCORES)], axis=1
    )
    return np.ascontiguousarray(cols.T).reshape(BSZ, SL, MP)


def kernel(repres, max_att, weight, **kw):
    from concourse.bass_utils import run_bass_kernel_spmd

    nc = get_nc()
    in_maps = make_in_maps(repres, max_att, weight)
    res = run_bass_kernel_spmd(nc, in_maps, list(range(NCORES)))
    return gather(res.results)


# revision 29
# speedup vs baseline: 1.0486x; 1.0486x over previous
"""Trainium2 Bass kernel for nn_AtteMatchLay (multi-perspective cosine matching).

Math (per flattened row n, perspective p):
    dot[n,p] = sum_d r[n,d]*m[n,d]*w2[p,d]
    n1s[n,p] = sum_d r[n,d]^2 * w2[p,d]        (w2 = weight**2)
    n2s[n,p] = sum_d m[n,d]^2 * w2[p,d]
    cos[n,p] = dot / (sqrt(n1s)*sqrt(n2s))

Strategy: data-parallel over the flattened N=16*512=8192 rows across 8 cores
(1024 rows each), contraction dim D on SBUF partitions (6 blocks of 128).

This kernel is DVE+ACT throughput-bound: the three elementwise products are
~13.5 engine-seconds over the two usable engines (GpSimd shares an
exclusive-lock SBUF port pair with DVE's 2-port mode and would stall it).
Everything is organized to (a) start those engines as early as possible,
(b) keep ops at full 1024 width for efficiency, (c) keep the post-stream
tail chain short.

  * r and m are packed interleaved into ONE DRAM tensor; each SP-queue
    trigger lands a matched (r,m) block pair (ACT queue must stay free:
    its sequencer serializes triggers against engine ops). Block 0 is the
    very first trigger so products start ~1.5us earlier; block 5 streams
    last as four quarter chunks so the final chain is short.
  * Products: rm+mm on DVE (2x bf16), rr on ACT (Square). Full width for
    b0..b4, quarter width for b5.
  * PSUM: two column groups x {dot,n1,n2} in their own banks (matmul
    start=True resets a whole bank, so groups never share banks).
  * Epilogue per group: u1,u2 = ARSQRT(n1s,n2s) on ACT (PSUM-direct,
    bf16 out), t = u1*u2 (DVE bf16 2x), cos = dot*t (bf16 out), DMA out.
    A dummy ARSQRT issued before any Square pins the one ACT table
    (abs_reciprocal_sqrt_and_small serves both) during the DMA-wait
    window instead of a 1.3us reload on the tail.
"""

import sys

if "/opt/trn_rl_repo" not in sys.path:
    sys.path.insert(0, "/opt/trn_rl_repo")

import numpy as np

# ---- problem constants (hardcoded per contract) ----
BSZ, SL, D, MP = 16, 512, 768, 20
N = BSZ * SL           # 8192 flattened rows
NCORES = 8
NSH = N // NCORES      # 1024 rows per core
P = 128                # SBUF partitions
NB = D // P            # 6 d-blocks
NBF = NB - 1           # blocks streamed full width
G = 2                  # PSUM column groups
GW = NSH // G          # 512
Q = 4                  # tail quarters of block 5
QW = NSH // Q          # 256

_CACHE = {}


def _build():
    import concourse.tile as tile
    from concourse import bacc, mybir

    f32 = mybir.dt.float32
    bf16 = mybir.dt.bfloat16
    nc = bacc.Bacc(None, target_bir_lowering=False)

    xD = nc.dram_tensor("xD", [P, 2 * NB * NSH], bf16, kind="ExternalInput")
    w2D = nc.dram_tensor("w2D", [P, NB * MP], bf16, kind="ExternalInput")
    out = nc.dram_tensor("out", [MP, NSH], bf16, kind="ExternalOutput")

    SQ = mybir.ActivationFunctionType.Square
    ARSQRT = mybir.ActivationFunctionType.Abs_reciprocal_sqrt
    MUL = mybir.AluOpType.mult

    with tile.TileContext(nc) as tc:
        with (
            tc.tile_pool(name="const", bufs=1) as const,
            tc.tile_pool(name="inp", bufs=1) as inp,
            tc.tile_pool(name="prod", bufs=3) as prod,
            tc.tile_pool(name="epi", bufs=1) as epi,
            tc.tile_pool(name="psum", bufs=1, space="PSUM") as psum,
        ):
            w2_sb = const.tile([P, NB, MP], bf16, tag="w2")
            x_sb = inp.tile([P, NB, 2, NSH], bf16, tag="x")
            bias_b = const.tile([P, 1], bf16, tag="bias_b")
            bias_f = const.tile([MP, 1], f32, tag="bias_f")
            dum = const.tile([MP, 1], f32, tag="dum")
            nc.gpsimd.memset(bias_b[:], 0.0)
            nc.gpsimd.memset(bias_f[:], 0.0)
            nc.gpsimd.memset(dum[:], 1.0)

            nc.scalar.activation(dum[:], dum[:], ARSQRT, bias=bias_f[:])

            # ---- DMA triggers (all SP queue, stream order) ----
            def ld(b, c0, c1):
                # chunk holds [r-cols | m-cols] of block b, columns c0:c1
                w = c1 - c0
                off = ld.off
                nc.sync.dma_start(
                    out=x_sb[:, b, :, c0:c1],
                    in_=xD[:, off : off + 2 * w].rearrange("p (t n) -> p t n", t=2),
                )
                ld.off = off + 2 * w

            ld.off = 0
            ld(0, 0, NSH)                       # block 0 first: earliest compute
            nc.sync.dma_start(
                out=w2_sb[:], in_=w2D[:, :].rearrange("p (b q) -> p b q", b=NB)
            )
            for b in range(1, NBF):
                ld(b, 0, NSH)
            # block 5 in column halves: 2KB descriptors (quarter-pair chunks
            # would drop to 1KB descriptors and halve the tail stream rate)
            ld(NB - 1, 0, GW)
            ld(NB - 1, GW, NSH)

            # ---- PSUM accumulators: per-group banks ----
            dot_ps, n1_ps, n2_ps = [], [], []
            for g in range(G):
                dps = psum.tile([MP, GW], f32, tag=f"dot{g}")
                n1p = psum.tile([MP, GW], f32, tag=f"n1{g}")
                n2p = psum.tile([MP, GW], f32, tag=f"n2{g}")
                dot_ps.append(dps)
                n1_ps.append(n1p)
                n2_ps.append(n2p)

            u1 = epi.tile([MP, NSH], bf16, tag="u1")
            u2 = epi.tile([MP, NSH], bf16, tag="u2")
            tt = epi.tile([MP, NSH], bf16, tag="tt")
            cos = epi.tile([MP, NSH], bf16, tag="cos")

            # ---- blocks 0..4: full-width products + 6 matmuls each ----
            for b in range(NBF):
                rsl = x_sb[:, b, 0, :]
                msl = x_sb[:, b, 1, :]
                rm = prod.tile([P, NSH], bf16, tag="rm")
                rr = prod.tile([P, NSH], bf16, tag="rr")
                mm = prod.tile([P, NSH], bf16, tag="mm")
                nc.vector.tensor_tensor(rm[:], rsl, msl, MUL)
                nc.scalar.activation(rr[:], rsl, SQ, bias=bias_b[:])
                nc.vector.tensor_tensor(mm[:], msl, msl, MUL)
                w2b = w2_sb[:, b, :]
                st = b == 0
                for g in range(G):
                    gsl = slice(g * GW, (g + 1) * GW)
                    nc.tensor.matmul(dot_ps[g][:], w2b, rm[:, gsl], start=st, stop=False)
                    nc.tensor.matmul(n1_ps[g][:], w2b, rr[:, gsl], start=st, stop=False)
                    nc.tensor.matmul(n2_ps[g][:], w2b, mm[:, gsl], start=st, stop=False)

            # ---- block 5 halves: products + finishing matmuls + epilogue ----
            rm5 = prod.tile([P, NSH], bf16, tag="rm5")
            rr5 = prod.tile([P, NSH], bf16, tag="rr5")
            mm5 = prod.tile([P, NSH], bf16, tag="mm5")
            w2b5 = w2_sb[:, NB - 1, :]
            b5 = NB - 1
            for g in range(G):
                gs = slice(g * GW, (g + 1) * GW)
                rq = x_sb[:, b5, 0, gs]
                mq = x_sb[:, b5, 1, gs]
                nc.vector.tensor_tensor(rm5[:, gs], rq, mq, MUL)
                nc.scalar.activation(rr5[:, gs], rq, SQ, bias=bias_b[:])
                nc.vector.tensor_tensor(mm5[:, gs], mq, mq, MUL)
                kw = dict(start=False, stop=True)
                nc.tensor.matmul(dot_ps[g][:], w2b5, rm5[:, gs], **kw)
                nc.tensor.matmul(n1_ps[g][:], w2b5, rr5[:, gs], **kw)
                nc.tensor.matmul(n2_ps[g][:], w2b5, mm5[:, gs], **kw)
                # cos = (dot*u1)*u2: the first DVE multiply overlaps ACT's
                # second arsqrt instead of waiting for it, and the final
                # multiply is cheap bf16 2x.
                nc.scalar.activation(u1[:, gs], n1_ps[g][:], ARSQRT, bias=bias_f[:])
                nc.scalar.activation(u2[:, gs], n2_ps[g][:], ARSQRT, bias=bias_f[:])
                nc.vector.tensor_tensor(tt[:, gs], dot_ps[g][:], u1[:, gs], MUL)
                nc.vector.tensor_tensor(cos[:, gs], tt[:, gs], u2[:, gs], MUL)
                nc.sync.dma_start(out=out[:, gs], in_=cos[:, gs])

    nc.finalize()
    return nc


def get_nc():
    if "nc" not in _CACHE:
        _CACHE["nc"] = _build()
    return _CACHE["nc"]


def _pack_pair(r2d, m2d):
    # [1024 rows, 768] f32 x2 -> [128, 12288] bf16 in stream-chunk order:
    # [r-b0|m-b0] ... [r-b4|m-b4], then block 5 as 4 quarter chunks.
    import ml_dtypes

    rt = r2d.T.reshape(NB, P, NSH)  # [b, p, n]
    mt = m2d.T.reshape(NB, P, NSH)
    parts = []
    for b in range(NBF):
        parts.append(rt[b])
        parts.append(mt[b])
    for g in range(G):
        gs = slice(g * GW, (g + 1) * GW)
        parts.append(rt[NB - 1][:, gs])
        parts.append(mt[NB - 1][:, gs])
    x = np.concatenate(parts, axis=1)
    return np.ascontiguousarray(x.astype(ml_dtypes.bfloat16))


def make_in_maps(repres, max_att, weight):
    import ml_dtypes

    r = np.ascontiguousarray(repres, dtype=np.float32).reshape(N, D)
    m = np.ascontiguousarray(max_att, dtype=np.float32).reshape(N, D)
    w2t = (weight.astype(np.float32) ** 2).T  # [D, MP]
    w2d = np.ascontiguousarray(
        w2t.reshape(NB, P, MP).transpose(1, 0, 2).reshape(P, NB * MP)
        .astype(ml_dtypes.bfloat16)
    )
    in_maps = []
    for c in range(NCORES):
        rows = slice(c * NSH, (c + 1) * NSH)
        in_maps.append({"xD": _pack_pair(r[rows], m[rows]), "w2D": w2d})
    return in_maps


def gather(results):
    cols = np.concatenate(
        [results[c]["out"].astype(np.float32) for c in range(NCORES)], axis=1
    )
    return np.ascontiguousarray(cols.T).reshape(BSZ, SL, MP)


def kernel(repres, max_att, weight, **kw):
    from concourse.bass_utils import run_bass_kernel_spmd

    nc = get_nc()
    in_maps = make_in_maps(repres, max_att, weight)
    res = run_bass_kernel_spmd(nc, in_maps, list(range(NCORES)))
    return gather(res.results)
